# revision 42
# baseline (speedup 1.0000x reference)
"""TRN2 Bass kernel for nn_Attention_5720896438407 (8-core data-parallel).

Mathematical collapse: the module computes SDPA over the *head* axis with a
single KV head (KV=1), so the softmax runs over a size-1 axis and every
attention weight is exactly 1.0.  The q path (q_a/q_norm/q_b), both rotary
embeddings, the nope/rope blend and the attention mask all cancel out, and
the module reduces to

    T  = hidden @ kv_a_w.T + kv_a_b                    # (ntok, 512)
    s  = rsqrt(mean(T^2, -1) + eps)                    # per-token RMS scale
    V  = (s*T) @ Wv'.T + kv_b_b[128:]                  # Wv' = kv_b_w[128:]*(1+kv_norm_w)
    Y  = V @ M.T      with  M = o_w.reshape(2048, 16, 128).sum(1)

Two further structural reductions (fast path, used when kv_b_b == 0):

1.  The row scaling s commutes through both remaining matmuls, so V's
    unscaled pre-image U = T @ Wv'.T is computed DIRECTLY from X as
    U.T = G @ X.T with G = Wv' @ kv_a_w precomputed on the host.  T is
    then needed ONLY for its row norm (the RMS statistic); the transpose
    pipeline of earlier versions disappears entirely, and s is applied in
    the final PSUM->SBUF copies of Y (tokens on partitions there).

2.  Because T only feeds a mean of 512 squares, independent per-element
    quantization errors average down by ~sqrt(512): step 1 runs in
    fp8-e4m3 with the DoubleRow perf mode (2x matmul rate, half the
    weight bytes) at a ~0.2% cost on s — invisible next to the 2e-2
    budget.  X is cast to fp8 on-chip by the otherwise-idle GPSIMD
    engine; the weights ship as fp8 (64x prescaled into e4m3's normal
    range, undone via the Square activation's input scale).

Distribution: pure data-parallel over the 8192 tokens — 1024 tokens per
NeuronCore, no collectives; 8 slabs of 128 tokens per core.  The kernel is
HBM-bound: ~6MB in (X fp16 4MB + fp8 weights + G/M), 4MB out (Y fp16).
Inputs stream on the SP HWDGE ring (w8/X-pair0 quarters interleaved, then
1MB X pairs), G/M and all outputs ride the ACT ring.  Zero-operand junk
matmuls keep the PE HAM clock gate open through the DMA-paced prologue.
End-to-end error vs the fp32 reference is ~1e-3 relative.
"""
import sys

sys.path.insert(0, "/opt/trn_rl_repo")

import numpy as np
import ml_dtypes
import concourse.bass as bass
import concourse.tile as tile
from concourse import bacc, mybir
from concourse.bass_utils import run_bass_kernel_spmd
from concourse.masks import make_identity

F32 = mybir.dt.float32
F16 = mybir.dt.float16
F8 = mybir.dt.float8e4
NP_F8 = ml_dtypes.float8_e4m3
DR = mybir.MatmulPerfMode.DoubleRow

HID = 2048
KV = 512
D = 128
OUT = 2048
EPS = 1e-6
N_HID_CK = HID // 128   # 16
N_DR = HID // 256       # 8 DoubleRow blocks
N_OUT_T = OUT // 512    # 4
SLAB = 128              # tokens per slab
N_CORES = 8
W8_SCALE = 64.0         # host prescale of kv_a_w into e4m3 range
AF = mybir.ActivationFunctionType

PREFIX_JUNK = 40        # junk matmuls before any real work
S2P_JUNK = (1, 0)       # junk per step-2 chunk for pairs 0 / 1

_NC_CACHE = {}


def _build_nc_fast(tok, with_ba):
    nslab = tok // SLAB
    assert tok % SLAB == 0 and nslab % 2 == 0 and nslab >= 4

    nc = bacc.Bacc("TRN2", target_bir_lowering=False, debug=False,
                   num_devices=1)

    # token slabs, pair-major: [pair, hid_row, hid_chunk, sub_slab, token]
    # (chunk-major: a chunk-quarter DMA is one contiguous 2KB run per
    # partition instead of 2x1KB descriptors)
    xts_d = nc.dram_tensor("xts", (nslab // 2, 128, N_HID_CK, 2, SLAB), F16,
                           kind="ExternalInput").ap()
    # 64*kv_a_w, DoubleRow-packed: [hid_row, dr_block, k_tile, kv]
    w8_d = nc.dram_tensor("w8", (128, N_DR, 2, KV), F8,
                          kind="ExternalInput").ap()
    # (Wv' @ kv_a_w).T chunks: [hid_row, hid_chunk, d]
    gt_d = nc.dram_tensor("gt", (128, N_HID_CK, D), F16,
                          kind="ExternalInput").ap()
    mt_d = nc.dram_tensor("mt", (D, OUT), F16, kind="ExternalInput").ap()
    if with_ba:
        ba_d = nc.dram_tensor("bar", (1, KV), F16, kind="ExternalInput").ap()
        onesr_d = nc.dram_tensor("onesr", (1, 128), F16,
                                 kind="ExternalInput").ap()
        cu_d = nc.dram_tensor("cu", (D, 1), F32, kind="ExternalInput").ap()
    y_d = nc.dram_tensor("y", (tok, OUT), F16, kind="ExternalOutput").ap()

    with tile.TileContext(nc) as tc:
        with tc.tile_pool(name="consts", bufs=1) as consts, \
             tc.tile_pool(name="pairs", bufs=nslab // 2) as pairs, \
             tc.tile_pool(name="x8p", bufs=4) as x8p, \
             tc.tile_pool(name="work", bufs=2) as work, \
             tc.tile_pool(name="ps_t", bufs=2, space="PSUM") as ps_t, \
             tc.tile_pool(name="ps_v", bufs=2, space="PSUM") as ps_v, \
             tc.tile_pool(name="ps_y", bufs=2, space="PSUM") as ps_y:
            # zero tile + PSUM scratch for HAM warm-up junk matmuls
            js = consts.tile([128, 128], F16, tag="js")
            nc.vector.memset(js[:], 0.0)
            junka = ps_y.tile([128, 1024], F32, tag="py", name="junka")
            junkb = ps_y.tile([128, 1024], F32, tag="py", name="junkb")
            jstate = [0]

            def junk_mm(n=1):
                for _ in range(n):
                    t = junka if jstate[0] % 2 == 0 else junkb
                    jstate[0] += 1
                    nc.tensor.matmul(t[:, 0:128], js[:], js[:],
                                     start=True, stop=True)

            # ---- input stream on the SP ring: X pair 0 (quartered, feeds
            #      the earliest step-2 work), the fp8 weights in halves,
            #      then the remaining 1MB pairs ----
            w8_s = consts.tile([128, N_DR, 2, KV], F8, tag="w8")
            pair_tiles = []
            p0 = pairs.tile([128, N_HID_CK, 2, SLAB], F16, tag="pair",
                            name="pair0")
            pair_tiles.append(p0)
            for h in range(4):
                ck = slice(4 * h, 4 * h + 4)
                nc.sync.dma_start(p0[:, ck, :, :], xts_d[0, :, ck, :, :])
            for h in range(2):
                nc.sync.dma_start(w8_s[:, 4 * h:4 * h + 4, :, :],
                                  w8_d[:, 4 * h:4 * h + 4, :, :])
            for p in range(1, nslab // 2):
                t = pairs.tile([128, N_HID_CK, 2, SLAB], F16, tag="pair",
                               name=f"pair{p}")
                nc.sync.dma_start(t[:], xts_d[p])
                pair_tiles.append(t)

            # ---- G/M (+ba consts) early on the ACT ring; outputs follow ----
            gt_s = consts.tile([128, N_HID_CK, D], F16, tag="gt")
            nc.scalar.dma_start(gt_s[:], gt_d)
            mt_s = consts.tile([128, OUT], F16, tag="mt")
            nc.scalar.dma_start(mt_s[:], mt_d)
            if with_ba:
                ba_s = consts.tile([1, KV], F16, tag="ba")
                nc.scalar.dma_start(ba_s[:], ba_d)
                onesr_s = consts.tile([1, 128], F16, tag="onesr")
                nc.scalar.dma_start(onesr_s[:], onesr_d)
                cu_s = consts.tile([128, 1], F32, tag="cu")
                nc.scalar.dma_start(cu_s[:], cu_d)
            eps_s = consts.tile([128, 1], F32, tag="eps")
            nc.vector.memset(eps_s[:], EPS)

            # on-chip fp8 cast of one X pair (vector engine); two
            # sub-slab copies permute chunk-major input into the
            # DoubleRow-friendly [sub_slab, chunk] layout
            def cast8(p):
                t = x8p.tile([128, 2, N_HID_CK, SLAB], F8, tag="x8",
                             name=f"x8_{p}")
                for j in range(2):
                    nc.vector.tensor_copy(t[:, j], pair_tiles[p][:, :, j, :])
                return t

            def step1dr(g, x8pair):
                # T.T accumulation in fp8 DoubleRow: 8 matmuls contracting
                # 256 hid rows each.  pt holds 64*T (the w8 prescale),
                # undone by the Square input scale below.
                j = g % 2
                pt = ps_t.tile([128, KV], F32, tag="pt", name=f"pt{g}")
                for m in range(N_DR):
                    nc.tensor.matmul(
                        pt[:], x8pair[:, j, 2 * m:2 * m + 2, :], w8_s[:, m],
                        start=(m == 0),
                        stop=(m == N_DR - 1 and not with_ba),
                        perf_mode=DR,
                    )
                if with_ba:
                    # rank-1 row-broadcast of 64*kv_a_b into the accumulation
                    nc.tensor.matmul(pt[:], onesr_s[:], ba_s[:],
                                     start=False, stop=True)
                return pt

            def step2pair(p, junk_per_chunk=0):
                # U.T = G @ X.T for both slabs of a pair at once: the pair
                # tile's [2, token] free dims give a 256-wide moving operand
                vtp = ps_v.tile([128, 2, SLAB], F32, tag="vtp",
                                name=f"vtp{p}")
                for ck in range(N_HID_CK):
                    nc.tensor.matmul(vtp[:], gt_s[:, ck, :],
                                     pair_tiles[p][:, ck, :, :],
                                     start=(ck == 0),
                                     stop=(ck == N_HID_CK - 1))
                    junk_mm(junk_per_chunk)
                return vtp

            def stat(g, pt):
                # RMS statistics; nothing downstream waits on this chain
                # except the final Y copies.  scale=1/64 undoes the w8
                # prescale inside the Square.
                sqj = work.tile([128, KV], F32, tag="sqj")
                ssq = work.tile([128, 1], F32, tag="ssq")
                nc.scalar.activation(sqj[:], pt[:], AF.Square,
                                     accum_out=ssq[:], scale=1.0 / W8_SCALE)
                rt = work.tile([128, 1], F32, tag="rt")
                nc.scalar.activation(rt[:], ssq[:], AF.Sqrt,
                                     bias=eps_s[:], scale=1.0 / KV)
                sc = work.tile([128, 1], F32, tag="sc", bufs=4)
                nc.vector.reciprocal(sc[:], rt[:])
                return sc

            def vts_pair(p, vtp):
                vts = work.tile([128, 2, SLAB], F16, tag="vts", bufs=2)
                if with_ba:
                    # U.T gains the constant column Wv'@kv_a_b (per-partition)
                    nc.scalar.activation(vts[:], vtp[:], AF.Identity,
                                         bias=cu_s[:], scale=1.0)
                else:
                    nc.vector.tensor_copy(vts[:], vtp[:])
                return vts

            def tail4(g, vts_ap, sc, swap=False):
                t0 = g * SLAB
                last = g == nslab - 1
                # step 4: Y = V @ M.T with the RMS scale applied in the
                # PSUM->SBUF copies (tokens are partitions here).  The four
                # matmuls write two double-bank PSUM tiles so the copies run
                # as two 1024-wide ops, one on each of vector/scalar.
                ysb = work.tile([128, OUT], F16, tag="ysb", bufs=6)
                pys = []
                for n in range(N_OUT_T):
                    if n % 2 == 0:
                        py = ps_y.tile([128, 1024], F32, tag="py",
                                       name=f"py{g}_{n // 2}")
                        pys.append(py)
                    nc.tensor.matmul(pys[-1][:, (n % 2) * 512:
                                              (n % 2) * 512 + 512],
                                     vts_ap,
                                     mt_s[:, n * 512:(n + 1) * 512],
                                     start=True, stop=True)
                for h in range(2):
                    ysl = ysb[:, h * 1024:(h + 1) * 1024]
                    if (h == 1) != swap:
                        nc.scalar.activation(ysl, pys[h][:], AF.Copy,
                                             bias=0.0, scale=sc[:])
                    else:
                        nc.vector.tensor_scalar_mul(ysl, pys[h][:], sc[:])
                    if last:
                        # ship each half as soon as its copy lands
                        nc.sync.dma_start(
                            y_d[t0:t0 + SLAB, h * 1024:(h + 1) * 1024], ysl)
                if not last:
                    nc.sync.dma_start(y_d[t0:t0 + SLAB, :], ysb[:])

            # ---- schedule (pair-granular, tails one pair behind, fp8
            #      casts prefetched one pair ahead) ----
            junk_mm(PREFIX_JUNK)
            # pair 0: step-2 chunks ride the X trickle (junk keeps the HAM
            # clock gate open); its fp8 step-1s run the moment w8 lands.
            vtp0 = step2pair(0, junk_per_chunk=S2P_JUNK[0])
            x8 = [cast8(0)]
            pt = step1dr(0, x8[0])
            sc0 = stat(0, pt)
            pt = step1dr(1, x8[0])
            sc1 = stat(1, pt)
            x8.append(cast8(1))
            vtp1 = step2pair(1, junk_per_chunk=S2P_JUNK[1])
            pt = step1dr(2, x8[1])
            sc2 = stat(2, pt)
            pt = step1dr(3, x8[1])
            sc3 = stat(3, pt)
            vts0 = vts_pair(0, vtp0)
            vts1 = vts_pair(1, vtp1)
            tail4(0, vts0[:, 0, :], sc0)
            tail4(1, vts0[:, 1, :], sc1)
            pend = [(2, vts1, 0, sc2), (3, vts1, 1, sc3)]
            if nslab // 2 > 2:
                x8.append(cast8(2))
            for p in range(2, nslab // 2):
                vtp = step2pair(p)
                pt = step1dr(2 * p, x8[p])
                sca = stat(2 * p, pt)
                pt = step1dr(2 * p + 1, x8[p])
                scb = stat(2 * p + 1, pt)
                vts = vts_pair(p, vtp)
                for (g, v, j, sc) in pend:
                    tail4(g, v[:, j, :], sc)
                if p + 1 < nslab // 2:
                    x8.append(cast8(p + 1))
                pend = [(2 * p, vts, 0, sca), (2 * p + 1, vts, 1, scb)]
            for i, (g, v, j, sc) in enumerate(pend):
                tail4(g, v[:, j, :], sc, swap=(i == 1))

    nc.compile()
    return nc


def _build_nc_legacy(tok, with_ba):
    """Fallback used when kv_b_b != 0 (never hit by the reference setup):
    fp16 step-1 + transpose pipeline with the V bias applied pre-scale."""
    nslab = tok // SLAB
    assert tok % SLAB == 0 and nslab % 2 == 0 and nslab >= 4
    N_KV_CK = KV // 128

    nc = bacc.Bacc("TRN2", target_bir_lowering=False, debug=False,
                   num_devices=1)

    xts_d = nc.dram_tensor("xts", (nslab // 2, 128, 2, N_HID_CK, SLAB), F16,
                           kind="ExternalInput").ap()
    w1s_d = nc.dram_tensor("w1s", (128, N_HID_CK, KV), F16,
                           kind="ExternalInput").ap()
    wvt_d = nc.dram_tensor("wvt", (KV, D), F16, kind="ExternalInput").ap()
    mt_d = nc.dram_tensor("mt", (D, OUT), F16, kind="ExternalInput").ap()
    bv_d = nc.dram_tensor("bv", (D, 1), F32, kind="ExternalInput").ap()
    if with_ba:
        ba_d = nc.dram_tensor("bar", (1, KV), F16, kind="ExternalInput").ap()
        onesr_d = nc.dram_tensor("onesr", (1, 128), F16,
                                 kind="ExternalInput").ap()
    y_d = nc.dram_tensor("y", (tok, OUT), F16, kind="ExternalOutput").ap()

    with tile.TileContext(nc) as tc:
        with tc.tile_pool(name="consts", bufs=1) as consts, \
             tc.tile_pool(name="pairs", bufs=nslab // 2) as pairs, \
             tc.tile_pool(name="work", bufs=2) as work, \
             tc.tile_pool(name="ps_t", bufs=2, space="PSUM") as ps_t, \
             tc.tile_pool(name="ps_r", bufs=1, space="PSUM") as ps_r, \
             tc.tile_pool(name="ps_v", bufs=1, space="PSUM") as ps_v, \
             tc.tile_pool(name="ps_y", bufs=4, space="PSUM") as ps_y:
            js = consts.tile([128, 128], F16, tag="js")
            nc.vector.memset(js[:], 0.0)
            junka = ps_y.tile([128, 512], F32, tag="py", name="junka")
            junkb = ps_y.tile([128, 512], F32, tag="py", name="junkb")
            jstate = [0]

            def junk_mm(n=1):
                for _ in range(n):
                    t = junka if jstate[0] % 2 == 0 else junkb
                    jstate[0] += 1
                    nc.tensor.matmul(t[:, 0:128], js[:], js[:],
                                     start=True, stop=True)

            w1_s = consts.tile([128, N_HID_CK, KV], F16, tag="w1")
            pair_tiles = []
            p0 = pairs.tile([128, 2, N_HID_CK, SLAB], F16, tag="pair",
                            name="pair0")
            pair_tiles.append(p0)
            for h in range(4):
                ck = slice(4 * h, 4 * h + 4)
                nc.sync.dma_start(w1_s[:, ck, :], w1s_d[:, ck, :])
                nc.sync.dma_start(p0[:, :, ck, :], xts_d[0, :, :, ck, :])
            for p in range(1, nslab // 2):
                t = pairs.tile([128, N_HID_CK, 2, SLAB], F16, tag="pair",
                               name=f"pair{p}")
                nc.sync.dma_start(t[:], xts_d[p])
                pair_tiles.append(t)

            def sg(g):
                p, j = divmod(g, 2)
                return pair_tiles[p][:, j]

            wv_s = []
            for c in range(N_KV_CK):
                t = consts.tile([128, D], F16, tag=f"wv_{c}", name=f"wv_{c}")
                nc.scalar.dma_start(t[:], wvt_d[c * 128:(c + 1) * 128, :])
                wv_s.append(t)
            mt_s = consts.tile([128, OUT], F16, tag="mt")
            nc.scalar.dma_start(mt_s[:], mt_d)
            bv_s = consts.tile([128, 1], F32, tag="bv")
            nc.scalar.dma_start(bv_s[:], bv_d)
            if with_ba:
                ba_s = consts.tile([1, KV], F16, tag="ba")
                nc.scalar.dma_start(ba_s[:], ba_d)
                onesr_s = consts.tile([1, 128], F16, tag="onesr")
                nc.scalar.dma_start(onesr_s[:], onesr_d)

            ident = consts.tile([128, 128], F16, tag="ident")
            make_identity(nc, ident[:])
            eps_s = consts.tile([128, 1], F32, tag="eps")
            nc.vector.memset(eps_s[:], EPS)

            def step1(g, junk_per_chunk=0):
                pt = ps_t.tile([128, KV], F32, tag="pt", name=f"pt{g}")
                for ck in range(N_HID_CK):
                    nc.tensor.matmul(
                        pt[:], sg(g)[:, ck, :], w1_s[:, ck, :],
                        start=(ck == 0),
                        stop=(ck == N_HID_CK - 1 and not with_ba),
                    )
                    junk_mm(junk_per_chunk)
                if with_ba:
                    nc.tensor.matmul(pt[:], onesr_s[:], ba_s[:],
                                     start=False, stop=True)
                return pt

            def stat(g, pt):
                sqj = work.tile([128, KV], F32, tag="sqj")
                ssq = work.tile([128, 1], F32, tag="ssq")
                nc.scalar.activation(sqj[:], pt[:], AF.Square,
                                     accum_out=ssq[:])
                rt = work.tile([128, 1], F32, tag="rt")
                nc.scalar.activation(rt[:], ssq[:], AF.Sqrt,
                                     bias=eps_s[:], scale=1.0 / KV)
                sc = work.tile([128, 1], F32, tag="sc", bufs=4)
                nc.vector.reciprocal(sc[:], rt[:])
                ttn = work.tile([128, KV], F16, tag="ttn", bufs=3)
                nc.vector.tensor_scalar_mul(ttn[:], pt[:], sc[:])
                return ttn, sc

            def tail_a(g, ttn):
                trp = ps_r.tile([128, N_KV_CK, SLAB], F16, tag="trp",
                                name=f"trp{g}")
                for c in range(N_KV_CK):
                    nc.tensor.transpose(trp[:, c, :],
                                        ttn[:, c * 128:(c + 1) * 128],
                                        ident[:])
                ttr = work.tile([128, N_KV_CK, SLAB], F16, tag="ttr", bufs=3)
                nc.vector.tensor_copy(ttr[:], trp[:])
                vtp = ps_v.tile([128, SLAB], F32, tag="vtp", name=f"vtp{g}")
                for c in range(N_KV_CK):
                    nc.tensor.matmul(vtp[:], wv_s[c][:], ttr[:, c, :],
                                     start=(c == 0),
                                     stop=(c == N_KV_CK - 1))
                vts = work.tile([128, SLAB], F16, tag="vts", bufs=3)
                nc.scalar.activation(vts[:], vtp[:], AF.Identity,
                                     bias=bv_s[:], scale=1.0)
                return vts

            def tail_b(g, vts, sc):
                t0 = g * SLAB
                last = g == nslab - 1
                ysb = work.tile([128, OUT], F16, tag="ysb", bufs=6)
                pys = []
                for n in range(N_OUT_T):
                    py = ps_y.tile([128, 512], F32, tag="py",
                                   name=f"py{g}_{n}")
                    nc.tensor.matmul(py[:], vts[:],
                                     mt_s[:, n * 512:(n + 1) * 512],
                                     start=True, stop=True)
                    pys.append(py)
                for n in range(N_OUT_T):
                    ysl = ysb[:, n * 512:(n + 1) * 512]
                    if n % 2 == 0:
                        nc.vector.tensor_copy(ysl, pys[n][:])
                    else:
                        nc.scalar.activation(ysl, pys[n][:], AF.Copy,
                                             bias=0.0, scale=1.0)
                    if last:
                        nc.scalar.dma_start(
                            y_d[t0:t0 + SLAB, n * 512:(n + 1) * 512], ysl)
                if not last:
                    nc.scalar.dma_start(y_d[t0:t0 + SLAB, :], ysb[:])

            junk_mm(PREFIX_JUNK)
            pt0 = step1(0, junk_per_chunk=2)
            pt1 = step1(1, junk_per_chunk=1)
            st0 = stat(0, pt0)
            st1 = stat(1, pt1)
            tail_b(0, tail_a(0, st0[0]), st0[1])
            tail_b(1, tail_a(1, st1[0]), st1[1])
            prev = None
            for g in range(2, nslab):
                pt = step1(g)
                if prev is not None:
                    pg, pttn, psc = prev
                    vts = tail_a(pg, pttn)
                    prev = (g,) + stat(g, pt)
                    tail_b(pg, vts, psc)
                else:
                    prev = (g,) + stat(g, pt)
            pg, pttn, psc = prev
            tail_b(pg, tail_a(pg, pttn), psc)

    nc.compile()
    return nc


def _host_prep(inputs):
    """Fold weights, swizzle X into fp16 token slabs, shard across cores."""
    h = np.asarray(inputs["hidden_states"], dtype=np.float32)
    b, s, hid = h.shape
    assert hid == HID
    x = np.ascontiguousarray(h.reshape(b * s, hid))
    ntok = b * s
    tok = ntok // N_CORES
    nslab = tok // SLAB

    kv_a_w = np.asarray(inputs["kv_a_w"], np.float32)
    kv_a_b = np.asarray(inputs["kv_a_b"], np.float32)
    kv_norm_w = np.asarray(inputs["kv_norm_w"], np.float32)
    kv_b_w = np.asarray(inputs["kv_b_w"], np.float32)
    kv_b_b = np.asarray(inputs["kv_b_b"], np.float32)
    o_w = np.asarray(inputs["o_w"], np.float32)

    wv = kv_b_w[D:2 * D] * (1.0 + kv_norm_w)[None, :]     # Wv' (128, 512)
    M = o_w.reshape(HID, 16, D).sum(axis=1)
    mt = np.ascontiguousarray(M.T).astype(np.float16)
    with_ba = bool(np.any(kv_a_b != 0.0))
    with_bv = bool(np.any(kv_b_b[D:2 * D] != 0.0))

    common = {"mt": mt}
    if with_bv:
        w1s = np.ascontiguousarray(
            kv_a_w.T.reshape(N_HID_CK, 128, KV).transpose(1, 0, 2)
        ).astype(np.float16)
        common["w1s"] = w1s
        common["wvt"] = np.ascontiguousarray(wv.T).astype(np.float16)
        common["bv"] = np.ascontiguousarray(
            kv_b_b[D:2 * D].reshape(D, 1)).astype(np.float32)
        if with_ba:
            common["bar"] = np.ascontiguousarray(
                kv_a_b.reshape(1, KV)).astype(np.float16)
            common["onesr"] = np.ones((1, 128), np.float16)
    else:
        # fast path: fp8 DoubleRow weights + fused G
        w8 = (W8_SCALE * kv_a_w.T).reshape(N_DR, 2, 128, KV)
        common["w8"] = np.ascontiguousarray(
            w8.transpose(2, 0, 1, 3)).astype(NP_F8)
        G = wv @ kv_a_w                                    # (128, 2048)
        common["gt"] = np.ascontiguousarray(
            G.T.reshape(N_HID_CK, 128, D).transpose(1, 0, 2)
        ).astype(np.float16)
        if with_ba:
            common["bar"] = np.ascontiguousarray(
                (W8_SCALE * kv_a_b).reshape(1, KV)).astype(np.float16)
            common["onesr"] = np.ones((1, 128), np.float16)
            common["cu"] = np.ascontiguousarray(
                (wv @ kv_a_b).reshape(D, 1)).astype(np.float32)

    in_maps = []
    perm = (2, 1, 3, 0, 4) if with_bv else (2, 1, 0, 3, 4)
    for i in range(N_CORES):
        shard = x[i * tok:(i + 1) * tok]
        # [pair, hid_row, sub_slab, hid_chunk, token] (legacy) or
        # [pair, hid_row, hid_chunk, sub_slab, token] (fast path)
        xts = np.ascontiguousarray(
            shard.T.reshape(N_HID_CK, 128, nslab // 2, 2, SLAB)
            .transpose(*perm)
        ).astype(np.float16)
        m = dict(common)
        m["xts"] = xts
        in_maps.append(m)

    def gather(results):
        y = np.concatenate([r["y"] for r in results], axis=0)
        return np.ascontiguousarray(y.reshape(b, s, HID).astype(np.float32))

    return in_maps, gather, with_ba, with_bv, tok


def _run(inputs, trace=False, **spmd_kwargs):
    in_maps, gather, with_ba, with_bv, tok = _host_prep(inputs)
    key = (tok, with_ba, with_bv)
    if key not in _NC_CACHE:
        builder = _build_nc_legacy if with_bv else _build_nc_fast
        _NC_CACHE[key] = builder(tok, with_ba)
    nc = _NC_CACHE[key]
    res = run_bass_kernel_spmd(nc, in_maps, core_ids=list(range(N_CORES)),
                               trace=trace, **spmd_kwargs)
    return gather(res.results), res


def kernel(**inputs) -> np.ndarray:
    y, _ = _run(inputs, trace=False)
    return y


# revision 43
# speedup vs baseline: 1.1698x; 1.1698x over previous
"""TRN2 Bass kernel for nn_Attention_5720896438407 (8-core data-parallel).

Mathematical collapse: the module computes SDPA over the *head* axis with a
single KV head (KV=1), so the softmax runs over a size-1 axis and every
attention weight is exactly 1.0.  The q path (q_a/q_norm/q_b), both rotary
embeddings, the nope/rope blend and the attention mask all cancel out, and
the module reduces to

    T  = hidden @ kv_a_w.T + kv_a_b                    # (ntok, 512)
    s  = rsqrt(mean(T^2, -1) + eps)                    # per-token RMS scale
    V  = (s*T) @ Wv'.T + kv_b_b[128:]                  # Wv' = kv_b_w[128:]*(1+kv_norm_w)
    Y  = V @ M.T      with  M = o_w.reshape(2048, 16, 128).sum(1)

Two further structural reductions (fast path, used when kv_b_b == 0):

1.  The row scaling s commutes through both remaining matmuls, so V's
    unscaled pre-image U = T @ Wv'.T is computed DIRECTLY from X as
    U.T = G @ X.T with G = Wv' @ kv_a_w precomputed on the host.  T is
    then needed ONLY for its row norm (the RMS statistic); the transpose
    pipeline of earlier versions disappears entirely, and s is applied in
    the final PSUM->SBUF copies of Y (tokens on partitions there).

2.  Because T only feeds a mean of 512 squares, independent per-element
    quantization errors average down by ~sqrt(512): step 1 runs in
    fp8-e4m3 with the DoubleRow perf mode (doubled contraction per
    matmul, half the weight bytes) at a ~0.2% cost on s — invisible next
    to the 2e-2 budget.  X is cast to fp8 on-chip by the vector engine
    (one pair ahead of use); the weights ship as fp8 (64x prescaled into
    e4m3's normal range, undone via the Square activation's input scale).
    DoubleRow's two k-tiles must be physically adjacent in the stationary
    operand, so the cast also permutes the chunk-major X into the
    [sub_slab, chunk] layout the DR matmuls read.

Distribution: pure data-parallel over the 8192 tokens — 1024 tokens per
NeuronCore, no collectives; 8 slabs of 128 tokens per core.  The kernel is
HBM-bound: ~6MB in (X fp16 4MB + fp8 weights + G/M), 4MB out (Y fp16).
Inputs stream on the SP HWDGE ring (X pair-0 chunk-quarters, fp8 weights,
then 1MB X pairs), G/M early and all outputs on the ACT ring.  Zero-operand
junk matmuls keep the PE HAM clock gate open through the DMA-paced
prologue.  End-to-end error vs the fp32 reference is ~2e-3 relative.
"""
import sys

sys.path.insert(0, "/opt/trn_rl_repo")

import numpy as np
import ml_dtypes
import concourse.bass as bass
import concourse.tile as tile
from concourse import bacc, mybir
from concourse.bass_utils import run_bass_kernel_spmd
from concourse.masks import make_identity

F32 = mybir.dt.float32
F16 = mybir.dt.float16
F8 = mybir.dt.float8e4
NP_F8 = ml_dtypes.float8_e4m3
DR = mybir.MatmulPerfMode.DoubleRow

HID = 2048
KV = 512
D = 128
OUT = 2048
EPS = 1e-6
N_HID_CK = HID // 128   # 16
N_DR = HID // 256       # 8 DoubleRow blocks
N_OUT_T = OUT // 512    # 4
SLAB = 128              # tokens per slab
N_CORES = 8
W8_SCALE = 64.0         # host prescale of kv_a_w into e4m3 range
AF = mybir.ActivationFunctionType

PREFIX_JUNK = 40        # junk matmuls before any real work
S2P_JUNK = (1, 0)       # junk per step-2 chunk for pairs 0 / 1

_NC_CACHE = {}


def _build_nc_fast(tok, with_ba):
    nslab = tok // SLAB
    assert tok % SLAB == 0 and nslab % 2 == 0 and nslab >= 4

    nc = bacc.Bacc("TRN2", target_bir_lowering=False, debug=False,
                   num_devices=1)

    # token slabs, pair-major: [pair, hid_row, hid_chunk, sub_slab, token]
    # (chunk-major: a chunk-quarter DMA is one contiguous 2KB run per
    # partition instead of 2x1KB descriptors)
    xts_d = nc.dram_tensor("xts", (nslab // 2, 128, N_HID_CK, 2, SLAB), F16,
                           kind="ExternalInput").ap()
    # 64*kv_a_w, DoubleRow-packed: [hid_row, dr_block, k_tile, kv]
    w8_d = nc.dram_tensor("w8", (128, N_DR, 2, KV), F8,
                          kind="ExternalInput").ap()
    # (Wv' @ kv_a_w).T chunks: [hid_row, hid_chunk, d]
    gt_d = nc.dram_tensor("gt", (128, N_HID_CK, D), F16,
                          kind="ExternalInput").ap()
    mt_d = nc.dram_tensor("mt", (D, OUT), F16, kind="ExternalInput").ap()
    if with_ba:
        ba_d = nc.dram_tensor("bar", (1, KV), F16, kind="ExternalInput").ap()
        onesr_d = nc.dram_tensor("onesr", (1, 128), F16,
                                 kind="ExternalInput").ap()
        cu_d = nc.dram_tensor("cu", (D, 1), F32, kind="ExternalInput").ap()
    y_d = nc.dram_tensor("y", (tok, OUT), F16, kind="ExternalOutput").ap()

    with tile.TileContext(nc) as tc:
        with tc.tile_pool(name="consts", bufs=1) as consts, \
             tc.tile_pool(name="pairs", bufs=nslab // 2) as pairs, \
             tc.tile_pool(name="x8p", bufs=4) as x8p, \
             tc.tile_pool(name="work", bufs=2) as work, \
             tc.tile_pool(name="ps_t", bufs=2, space="PSUM") as ps_t, \
             tc.tile_pool(name="ps_v", bufs=2, space="PSUM") as ps_v, \
             tc.tile_pool(name="ps_y", bufs=2, space="PSUM") as ps_y:
            # zero tile + PSUM scratch for HAM warm-up junk matmuls
            js = consts.tile([128, 128], F16, tag="js")
            nc.vector.memset(js[:], 0.0)
            junka = ps_y.tile([128, 1024], F32, tag="py", name="junka")
            junkb = ps_y.tile([128, 1024], F32, tag="py", name="junkb")
            jstate = [0]

            def junk_mm(n=1):
                for _ in range(n):
                    t = junka if jstate[0] % 2 == 0 else junkb
                    jstate[0] += 1
                    nc.tensor.matmul(t[:, 0:128], js[:], js[:],
                                     start=True, stop=True)

            # ---- input stream on the SP ring: X pair 0 (quartered, feeds
            #      the earliest step-2 work), the fp8 weights in halves,
            #      then the remaining 1MB pairs ----
            w8_s = consts.tile([128, N_DR, 2, KV], F8, tag="w8")
            pair_tiles = []
            p0 = pairs.tile([128, N_HID_CK, 2, SLAB], F16, tag="pair",
                            name="pair0")
            pair_tiles.append(p0)
            for h in range(4):
                ck = slice(4 * h, 4 * h + 4)
                nc.sync.dma_start(p0[:, ck, :, :], xts_d[0, :, ck, :, :])
            for h in range(2):
                nc.sync.dma_start(w8_s[:, 4 * h:4 * h + 4, :, :],
                                  w8_d[:, 4 * h:4 * h + 4, :, :])
            for p in range(1, nslab // 2):
                t = pairs.tile([128, N_HID_CK, 2, SLAB], F16, tag="pair",
                               name=f"pair{p}")
                nc.sync.dma_start(t[:], xts_d[p])
                pair_tiles.append(t)

            # ---- G/M (+ba consts) early on the ACT ring; outputs follow ----
            gt_s = consts.tile([128, N_HID_CK, D], F16, tag="gt")
            nc.scalar.dma_start(gt_s[:], gt_d)
            mt_s = consts.tile([128, OUT], F16, tag="mt")
            nc.scalar.dma_start(mt_s[:], mt_d)
            if with_ba:
                ba_s = consts.tile([1, KV], F16, tag="ba")
                nc.scalar.dma_start(ba_s[:], ba_d)
                onesr_s = consts.tile([1, 128], F16, tag="onesr")
                nc.scalar.dma_start(onesr_s[:], onesr_d)
                cu_s = consts.tile([128, 1], F32, tag="cu")
                nc.scalar.dma_start(cu_s[:], cu_d)
            eps_s = consts.tile([128, 1], F32, tag="eps")
            nc.vector.memset(eps_s[:], EPS)

            # on-chip fp8 cast of one X pair (vector engine); two
            # sub-slab copies permute chunk-major input into the
            # DoubleRow-friendly [sub_slab, chunk] layout
            def cast8(p):
                t = x8p.tile([128, 2, N_HID_CK, SLAB], F8, tag="x8",
                             name=f"x8_{p}")
                for j in range(2):
                    nc.vector.tensor_copy(t[:, j], pair_tiles[p][:, :, j, :])
                return t

            def step1dr(g, x8pair):
                # T.T accumulation in fp8 DoubleRow: 8 matmuls contracting
                # 256 hid rows each.  pt holds 64*T (the w8 prescale),
                # undone by the Square input scale below.
                j = g % 2
                pt = ps_t.tile([128, KV], F32, tag="pt", name=f"pt{g}")
                for m in range(N_DR):
                    nc.tensor.matmul(
                        pt[:], x8pair[:, j, 2 * m:2 * m + 2, :], w8_s[:, m],
                        start=(m == 0),
                        stop=(m == N_DR - 1 and not with_ba),
                        perf_mode=DR,
                    )
                if with_ba:
                    # rank-1 row-broadcast of 64*kv_a_b into the accumulation
                    nc.tensor.matmul(pt[:], onesr_s[:], ba_s[:],
                                     start=False, stop=True)
                return pt

            def step2pair(p, junk_per_chunk=0):
                # U.T = G @ X.T for both slabs of a pair at once: the pair
                # tile's [2, token] free dims give a 256-wide moving operand
                vtp = ps_v.tile([128, 2, SLAB], F32, tag="vtp",
                                name=f"vtp{p}")
                for ck in range(N_HID_CK):
                    nc.tensor.matmul(vtp[:], gt_s[:, ck, :],
                                     pair_tiles[p][:, ck, :, :],
                                     start=(ck == 0),
                                     stop=(ck == N_HID_CK - 1))
                    junk_mm(junk_per_chunk)
                return vtp

            def stat(g, pt):
                # RMS statistics; nothing downstream waits on this chain
                # except the final Y copies.  scale=1/64 undoes the w8
                # prescale inside the Square.
                sqj = work.tile([128, KV], F32, tag="sqj")
                ssq = work.tile([128, 1], F32, tag="ssq")
                nc.scalar.activation(sqj[:], pt[:], AF.Square,
                                     accum_out=ssq[:], scale=1.0 / W8_SCALE)
                rt = work.tile([128, 1], F32, tag="rt")
                nc.scalar.activation(rt[:], ssq[:], AF.Sqrt,
                                     bias=eps_s[:], scale=1.0 / KV)
                sc = work.tile([128, 1], F32, tag="sc", bufs=4)
                nc.vector.reciprocal(sc[:], rt[:])
                return sc

            def vts_pair(p, vtp):
                vts = work.tile([128, 2, SLAB], F16, tag="vts", bufs=2)
                if with_ba:
                    # U.T gains the constant column Wv'@kv_a_b (per-partition)
                    nc.scalar.activation(vts[:], vtp[:], AF.Identity,
                                         bias=cu_s[:], scale=1.0)
                else:
                    nc.vector.tensor_copy(vts[:], vtp[:])
                return vts

            def tail4(g, vts_ap, sc, swap=False):
                t0 = g * SLAB
                last = g == nslab - 1
                # step 4: Y = V @ M.T with the RMS scale applied in the
                # PSUM->SBUF copies (tokens are partitions here).  The four
                # matmuls write two double-bank PSUM tiles so the copies run
                # as two 1024-wide ops, one on each of vector/scalar.
                ysb = work.tile([128, OUT], F16, tag="ysb", bufs=6)
                pys = []
                for n in range(N_OUT_T):
                    if n % 2 == 0:
                        py = ps_y.tile([128, 1024], F32, tag="py",
                                       name=f"py{g}_{n // 2}")
                        pys.append(py)
                    nc.tensor.matmul(pys[-1][:, (n % 2) * 512:
                                              (n % 2) * 512 + 512],
                                     vts_ap,
                                     mt_s[:, n * 512:(n + 1) * 512],
                                     start=True, stop=True)
                for h in range(2):
                    ysl = ysb[:, h * 1024:(h + 1) * 1024]
                    if (h == 1) != swap:
                        nc.scalar.activation(ysl, pys[h][:], AF.Copy,
                                             bias=0.0, scale=sc[:])
                    else:
                        nc.vector.tensor_scalar_mul(ysl, pys[h][:], sc[:])
                    if last:
                        # ship each half as soon as its copy lands
                        nc.sync.dma_start(
                            y_d[t0:t0 + SLAB, h * 1024:(h + 1) * 1024], ysl)
                if not last:
                    nc.sync.dma_start(y_d[t0:t0 + SLAB, :], ysb[:])

            # ---- schedule (pair-granular, tails one pair behind, fp8
            #      casts prefetched one pair ahead) ----
            junk_mm(PREFIX_JUNK)
            # pair 0: step-2 chunks ride the X trickle (junk keeps the HAM
            # clock gate open); its fp8 step-1s run the moment w8 lands.
            vtp0 = step2pair(0, junk_per_chunk=S2P_JUNK[0])
            x8 = [cast8(0)]
            pt = step1dr(0, x8[0])
            sc0 = stat(0, pt)
            pt = step1dr(1, x8[0])
            sc1 = stat(1, pt)
            x8.append(cast8(1))
            vtp1 = step2pair(1, junk_per_chunk=S2P_JUNK[1])
            pt = step1dr(2, x8[1])
            sc2 = stat(2, pt)
            pt = step1dr(3, x8[1])
            sc3 = stat(3, pt)
            vts0 = vts_pair(0, vtp0)
            vts1 = vts_pair(1, vtp1)
            tail4(0, vts0[:, 0, :], sc0)
            tail4(1, vts0[:, 1, :], sc1)
            pend = [(2, vts1, 0, sc2), (3, vts1, 1, sc3)]
            if nslab // 2 > 2:
                x8.append(cast8(2))
            for p in range(2, nslab // 2):
                vtp = step2pair(p)
                pt = step1dr(2 * p, x8[p])
                sca = stat(2 * p, pt)
                pt = step1dr(2 * p + 1, x8[p])
                scb = stat(2 * p + 1, pt)
                vts = vts_pair(p, vtp)
                for (g, v, j, sc) in pend:
                    tail4(g, v[:, j, :], sc)
                if p + 1 < nslab // 2:
                    x8.append(cast8(p + 1))
                pend = [(2 * p, vts, 0, sca), (2 * p + 1, vts, 1, scb)]
            for i, (g, v, j, sc) in enumerate(pend):
                tail4(g, v[:, j, :], sc, swap=(i == 1))

    nc.compile()
    return nc


def _build_nc_legacy(tok, with_ba):
    """Fallback used when kv_b_b != 0 (never hit by the reference setup):
    fp16 step-1 + transpose pipeline with the V bias applied pre-scale."""
    nslab = tok // SLAB
    assert tok % SLAB == 0 and nslab % 2 == 0 and nslab >= 4
    N_KV_CK = KV // 128

    nc = bacc.Bacc("TRN2", target_bir_lowering=False, debug=False,
                   num_devices=1)

    xts_d = nc.dram_tensor("xts", (nslab // 2, 128, 2, N_HID_CK, SLAB), F16,
                           kind="ExternalInput").ap()
    w1s_d = nc.dram_tensor("w1s", (128, N_HID_CK, KV), F16,
                           kind="ExternalInput").ap()
    wvt_d = nc.dram_tensor("wvt", (KV, D), F16, kind="ExternalInput").ap()
    mt_d = nc.dram_tensor("mt", (D, OUT), F16, kind="ExternalInput").ap()
    bv_d = nc.dram_tensor("bv", (D, 1), F32, kind="ExternalInput").ap()
    if with_ba:
        ba_d = nc.dram_tensor("bar", (1, KV), F16, kind="ExternalInput").ap()
        onesr_d = nc.dram_tensor("onesr", (1, 128), F16,
                                 kind="ExternalInput").ap()
    y_d = nc.dram_tensor("y", (tok, OUT), F16, kind="ExternalOutput").ap()

    with tile.TileContext(nc) as tc:
        with tc.tile_pool(name="consts", bufs=1) as consts, \
             tc.tile_pool(name="pairs", bufs=nslab // 2) as pairs, \
             tc.tile_pool(name="work", bufs=2) as work, \
             tc.tile_pool(name="ps_t", bufs=2, space="PSUM") as ps_t, \
             tc.tile_pool(name="ps_r", bufs=1, space="PSUM") as ps_r, \
             tc.tile_pool(name="ps_v", bufs=1, space="PSUM") as ps_v, \
             tc.tile_pool(name="ps_y", bufs=4, space="PSUM") as ps_y:
            js = consts.tile([128, 128], F16, tag="js")
            nc.vector.memset(js[:], 0.0)
            junka = ps_y.tile([128, 512], F32, tag="py", name="junka")
            junkb = ps_y.tile([128, 512], F32, tag="py", name="junkb")
            jstate = [0]

            def junk_mm(n=1):
                for _ in range(n):
                    t = junka if jstate[0] % 2 == 0 else junkb
                    jstate[0] += 1
                    nc.tensor.matmul(t[:, 0:128], js[:], js[:],
                                     start=True, stop=True)

            w1_s = consts.tile([128, N_HID_CK, KV], F16, tag="w1")
            pair_tiles = []
            p0 = pairs.tile([128, 2, N_HID_CK, SLAB], F16, tag="pair",
                            name="pair0")
            pair_tiles.append(p0)
            for h in range(4):
                ck = slice(4 * h, 4 * h + 4)
                nc.sync.dma_start(w1_s[:, ck, :], w1s_d[:, ck, :])
                nc.sync.dma_start(p0[:, :, ck, :], xts_d[0, :, :, ck, :])
            for p in range(1, nslab // 2):
                t = pairs.tile([128, N_HID_CK, 2, SLAB], F16, tag="pair",
                               name=f"pair{p}")
                nc.sync.dma_start(t[:], xts_d[p])
                pair_tiles.append(t)

            def sg(g):
                p, j = divmod(g, 2)
                return pair_tiles[p][:, j]

            wv_s = []
            for c in range(N_KV_CK):
                t = consts.tile([128, D], F16, tag=f"wv_{c}", name=f"wv_{c}")
                nc.scalar.dma_start(t[:], wvt_d[c * 128:(c + 1) * 128, :])
                wv_s.append(t)
            mt_s = consts.tile([128, OUT], F16, tag="mt")
            nc.scalar.dma_start(mt_s[:], mt_d)
            bv_s = consts.tile([128, 1], F32, tag="bv")
            nc.scalar.dma_start(bv_s[:], bv_d)
            if with_ba:
                ba_s = consts.tile([1, KV], F16, tag="ba")
                nc.scalar.dma_start(ba_s[:], ba_d)
                onesr_s = consts.tile([1, 128], F16, tag="onesr")
                nc.scalar.dma_start(onesr_s[:], onesr_d)

            ident = consts.tile([128, 128], F16, tag="ident")
            make_identity(nc, ident[:])
            eps_s = consts.tile([128, 1], F32, tag="eps")
            nc.vector.memset(eps_s[:], EPS)

            def step1(g, junk_per_chunk=0):
                pt = ps_t.tile([128, KV], F32, tag="pt", name=f"pt{g}")
                for ck in range(N_HID_CK):
                    nc.tensor.matmul(
                        pt[:], sg(g)[:, ck, :], w1_s[:, ck, :],
                        start=(ck == 0),
                        stop=(ck == N_HID_CK - 1 and not with_ba),
                    )
                    junk_mm(junk_per_chunk)
                if with_ba:
                    nc.tensor.matmul(pt[:], onesr_s[:], ba_s[:],
                                     start=False, stop=True)
                return pt

            def stat(g, pt):
                sqj = work.tile([128, KV], F32, tag="sqj")
                ssq = work.tile([128, 1], F32, tag="ssq")
                nc.scalar.activation(sqj[:], pt[:], AF.Square,
                                     accum_out=ssq[:])
                rt = work.tile([128, 1], F32, tag="rt")
                nc.scalar.activation(rt[:], ssq[:], AF.Sqrt,
                                     bias=eps_s[:], scale=1.0 / KV)
                sc = work.tile([128, 1], F32, tag="sc", bufs=4)
                nc.vector.reciprocal(sc[:], rt[:])
                ttn = work.tile([128, KV], F16, tag="ttn", bufs=3)
                nc.vector.tensor_scalar_mul(ttn[:], pt[:], sc[:])
                return ttn, sc

            def tail_a(g, ttn):
                trp = ps_r.tile([128, N_KV_CK, SLAB], F16, tag="trp",
                                name=f"trp{g}")
                for c in range(N_KV_CK):
                    nc.tensor.transpose(trp[:, c, :],
                                        ttn[:, c * 128:(c + 1) * 128],
                                        ident[:])
                ttr = work.tile([128, N_KV_CK, SLAB], F16, tag="ttr", bufs=3)
                nc.vector.tensor_copy(ttr[:], trp[:])
                vtp = ps_v.tile([128, SLAB], F32, tag="vtp", name=f"vtp{g}")
                for c in range(N_KV_CK):
                    nc.tensor.matmul(vtp[:], wv_s[c][:], ttr[:, c, :],
                                     start=(c == 0),
                                     stop=(c == N_KV_CK - 1))
                vts = work.tile([128, SLAB], F16, tag="vts", bufs=3)
                nc.scalar.activation(vts[:], vtp[:], AF.Identity,
                                     bias=bv_s[:], scale=1.0)
                return vts

            def tail_b(g, vts, sc):
                t0 = g * SLAB
                last = g == nslab - 1
                ysb = work.tile([128, OUT], F16, tag="ysb", bufs=6)
                pys = []
                for n in range(N_OUT_T):
                    py = ps_y.tile([128, 512], F32, tag="py",
                                   name=f"py{g}_{n}")
                    nc.tensor.matmul(py[:], vts[:],
                                     mt_s[:, n * 512:(n + 1) * 512],
                                     start=True, stop=True)
                    pys.append(py)
                for n in range(N_OUT_T):
                    ysl = ysb[:, n * 512:(n + 1) * 512]
                    if n % 2 == 0:
                        nc.vector.tensor_copy(ysl, pys[n][:])
                    else:
                        nc.scalar.activation(ysl, pys[n][:], AF.Copy,
                                             bias=0.0, scale=1.0)
                    if last:
                        nc.scalar.dma_start(
                            y_d[t0:t0 + SLAB, n * 512:(n + 1) * 512], ysl)
                if not last:
                    nc.scalar.dma_start(y_d[t0:t0 + SLAB, :], ysb[:])

            junk_mm(PREFIX_JUNK)
            pt0 = step1(0, junk_per_chunk=2)
            pt1 = step1(1, junk_per_chunk=1)
            st0 = stat(0, pt0)
            st1 = stat(1, pt1)
            tail_b(0, tail_a(0, st0[0]), st0[1])
            tail_b(1, tail_a(1, st1[0]), st1[1])
            prev = None
            for g in range(2, nslab):
                pt = step1(g)
                if prev is not None:
                    pg, pttn, psc = prev
                    vts = tail_a(pg, pttn)
                    prev = (g,) + stat(g, pt)
                    tail_b(pg, vts, psc)
                else:
                    prev = (g,) + stat(g, pt)
            pg, pttn, psc = prev
            tail_b(pg, tail_a(pg, pttn), psc)

    nc.compile()
    return nc


def _host_prep(inputs):
    """Fold weights, swizzle X into fp16 token slabs, shard across cores."""
    h = np.asarray(inputs["hidden_states"], dtype=np.float32)
    b, s, hid = h.shape
    assert hid == HID
    x = np.ascontiguousarray(h.reshape(b * s, hid))
    ntok = b * s
    tok = ntok // N_CORES
    nslab = tok // SLAB

    kv_a_w = np.asarray(inputs["kv_a_w"], np.float32)
    kv_a_b = np.asarray(inputs["kv_a_b"], np.float32)
    kv_norm_w = np.asarray(inputs["kv_norm_w"], np.float32)
    kv_b_w = np.asarray(inputs["kv_b_w"], np.float32)
    kv_b_b = np.asarray(inputs["kv_b_b"], np.float32)
    o_w = np.asarray(inputs["o_w"], np.float32)

    wv = kv_b_w[D:2 * D] * (1.0 + kv_norm_w)[None, :]     # Wv' (128, 512)
    M = o_w.reshape(HID, 16, D).sum(axis=1)
    mt = np.ascontiguousarray(M.T).astype(np.float16)
    with_ba = bool(np.any(kv_a_b != 0.0))
    with_bv = bool(np.any(kv_b_b[D:2 * D] != 0.0))

    common = {"mt": mt}
    if with_bv:
        w1s = np.ascontiguousarray(
            kv_a_w.T.reshape(N_HID_CK, 128, KV).transpose(1, 0, 2)
        ).astype(np.float16)
        common["w1s"] = w1s
        common["wvt"] = np.ascontiguousarray(wv.T).astype(np.float16)
        common["bv"] = np.ascontiguousarray(
            kv_b_b[D:2 * D].reshape(D, 1)).astype(np.float32)
        if with_ba:
            common["bar"] = np.ascontiguousarray(
                kv_a_b.reshape(1, KV)).astype(np.float16)
            common["onesr"] = np.ones((1, 128), np.float16)
    else:
        # fast path: fp8 DoubleRow weights + fused G
        w8 = (W8_SCALE * kv_a_w.T).reshape(N_DR, 2, 128, KV)
        common["w8"] = np.ascontiguousarray(
            w8.transpose(2, 0, 1, 3)).astype(NP_F8)
        G = wv @ kv_a_w                                    # (128, 2048)
        common["gt"] = np.ascontiguousarray(
            G.T.reshape(N_HID_CK, 128, D).transpose(1, 0, 2)
        ).astype(np.float16)
        if with_ba:
            common["bar"] = np.ascontiguousarray(
                (W8_SCALE * kv_a_b).reshape(1, KV)).astype(np.float16)
            common["onesr"] = np.ones((1, 128), np.float16)
            common["cu"] = np.ascontiguousarray(
                (wv @ kv_a_b).reshape(D, 1)).astype(np.float32)

    in_maps = []
    perm = (2, 1, 3, 0, 4) if with_bv else (2, 1, 0, 3, 4)
    for i in range(N_CORES):
        shard = x[i * tok:(i + 1) * tok]
        # [pair, hid_row, sub_slab, hid_chunk, token] (legacy) or
        # [pair, hid_row, hid_chunk, sub_slab, token] (fast path)
        xts = np.ascontiguousarray(
            shard.T.reshape(N_HID_CK, 128, nslab // 2, 2, SLAB)
            .transpose(*perm)
        ).astype(np.float16)
        m = dict(common)
        m["xts"] = xts
        in_maps.append(m)

    def gather(results):
        y = np.concatenate([r["y"] for r in results], axis=0)
        return np.ascontiguousarray(y.reshape(b, s, HID).astype(np.float32))

    return in_maps, gather, with_ba, with_bv, tok


def _run(inputs, trace=False, **spmd_kwargs):
    in_maps, gather, with_ba, with_bv, tok = _host_prep(inputs)
    key = (tok, with_ba, with_bv)
    if key not in _NC_CACHE:
        builder = _build_nc_legacy if with_bv else _build_nc_fast
        _NC_CACHE[key] = builder(tok, with_ba)
    nc = _NC_CACHE[key]
    res = run_bass_kernel_spmd(nc, in_maps, core_ids=list(range(N_CORES)),
                               trace=trace, **spmd_kwargs)
    return gather(res.results), res


def kernel(**inputs) -> np.ndarray:
    y, _ = _run(inputs, trace=False)
    return y
